# revision 5
# baseline (speedup 1.0000x reference)
"""Trainium2 Bass kernel for multi-head attention (B=2, S=2048, H=16, D=128).

Computes y = softmax(Q @ K^T / D) @ V per (batch, head) pair, returning
[B*S, H*D] float32.

Sharding: 32 (b, h) pairs across 8 cores, 4 pairs per core (tensor parallel
over heads, data parallel over batch). Each core computes full S x S
attention for its pairs. Host pre-transposes Q/K to [d, s] layout (d-major)
and casts Q/K/V to bf16 so the device kernel needs no input transposes.

Per-core dataflow per (pair, q-chunk of 512):
  - S^T[kpos, q] = K @ Q^T via PE matmuls (lhsT=K^T block, rhs=Q^T chunk),
    accumulated in PSUM in batches of 4/2 k-blocks (ping-ponged between two
    PSUM pools sized to fill the ACT pipe with 2048/1024-elem exp ops).
  - exp(S^T / 128) on the scalar engine (scale fused into the activation),
    PSUM -> SBUF, bf16 out. No max-subtraction: |scores/128| < ~0.5 for
    randn inputs, so exp is well-conditioned.
  - y^T[d, q] += matmul (lhsT=V block [kpos, d], rhs=exp block [kpos, q])
    accumulated over the 16 k-blocks in PSUM.
  - Softmax denominator: binary-tree sum of the 16 exp blocks on DVE (bf16,
    2x mode, first level starts mid-chunk), then a PE matmul against a
    ones-vector reduces the remaining 128 partitions -> denom per q (fp32).
  - y^T copied to SBUF (cast bf16), PE-transposed per 128x128 block to
    y[q, d], scaled by 1/denom (per-partition scalar on DVE), DMA'd out.

The scalar engine (exp over S^2 elements at 1 elem/cycle/lane) is the
roofline for this kernel; the schedule keeps it saturated.
"""

import numpy as np
import ml_dtypes

B, S, H, D = 2, 2048, 16, 128
N_CORES = 8
PAIRS = (B * H) // N_CORES  # 4 pairs per core
QC = 512                    # q-chunk size
NKB = S // 128              # 16 k-blocks per sequence
# k-block batches per q-chunk: sized so the two PSUM score pools (4 + 2
# banks) ping-pong while yT (1 bank) + aux (1 bank) fit in the 8 banks.
GROUPS = [[0, 1, 2, 3], [4, 5], [6, 7, 8, 9], [10, 11], [12, 13, 14, 15]]

_cache = {}


def _build(n_pairs, nqc):
    import concourse.bacc as bacc
    import concourse.tile as tile
    import concourse.mybir as mybir
    from concourse.masks import make_identity

    bf16 = mybir.dt.bfloat16
    f32 = mybir.dt.float32
    Exp = mybir.ActivationFunctionType.Exp

    nc = bacc.Bacc(None, target_bir_lowering=False, debug=False)
    qt = nc.dram_tensor("qt", [n_pairs, 128, S], bf16, kind="ExternalInput")
    kt = nc.dram_tensor("kt", [n_pairs, 128, S], bf16, kind="ExternalInput")
    vt = nc.dram_tensor("vt", [n_pairs, 128, NKB, 128], bf16, kind="ExternalInput")
    y = nc.dram_tensor("y", [n_pairs, S, 128], f32, kind="ExternalOutput")

    with tile.TileContext(nc) as tc:
        with (
            tc.tile_pool(name="const", bufs=1) as constp,
            tc.tile_pool(name="qts", bufs=2) as qtsp,
            tc.tile_pool(name="kts", bufs=2) as ktsp,
            tc.tile_pool(name="vs", bufs=2) as vsp,
            tc.tile_pool(name="es", bufs=2) as esp,
            tc.tile_pool(name="esum", bufs=2) as esump,
            tc.tile_pool(name="yts", bufs=2) as ytsp,
            tc.tile_pool(name="rall", bufs=2) as rallp,
            tc.tile_pool(name="yn", bufs=3) as ynp,
            tc.tile_pool(name="stA", bufs=1, space="PSUM") as stAp,
            tc.tile_pool(name="stB", bufs=1, space="PSUM") as stBp,
            tc.tile_pool(name="yT", bufs=1, space="PSUM") as yTp,
            tc.tile_pool(name="aux", bufs=1, space="PSUM") as auxp,
        ):
            ones = constp.tile([128, 1], bf16)
            nc.vector.memset(ones, 1.0)
            ident = constp.tile([128, 128], bf16)
            make_identity(nc, ident)

            def emit_A(j, qc, tiles):
                """Score matmuls + exp + y^T accumulation + yT copy + tree-sum."""
                qts, kts, vs = tiles["qkv"]
                es = esp.tile([128, NKB * QC], bf16, tag="es", name=f"es_{j}_{qc}")
                esum = esump.tile([128, NKB * QC // 4], bf16,
                                  tag="esum", name=f"esum_{j}_{qc}")
                yT = yTp.tile([128, QC], f32, tag="yT", name=f"yT_{j}_{qc}")
                q_sl = qts[:, qc * QC:(qc + 1) * QC]
                prev = None
                for gi, g in enumerate(GROUPS):
                    pool = stAp if len(g) == 4 else stBp
                    st = pool.tile([128, QC * len(g)], f32, tag="st",
                                   name=f"st_{j}_{qc}_{g[0]}")
                    for i, kb in enumerate(g):
                        nc.tensor.matmul(
                            st[:, i * QC:(i + 1) * QC],
                            lhsT=kts[:, kb * 128:(kb + 1) * 128],
                            rhs=q_sl,
                            start=True, stop=True,
                        )
                    # y-matmuls of the previous group keep PE busy while the
                    # scalar engine runs exp on this group.
                    if prev is not None:
                        for kb in prev:
                            nc.tensor.matmul(
                                yT,
                                lhsT=vs[:, kb * 128:(kb + 1) * 128],
                                rhs=es[:, kb * QC:(kb + 1) * QC],
                                start=(kb == 0), stop=(kb == NKB - 1),
                            )
                    nc.scalar.activation(
                        es[:, g[0] * QC:(g[-1] + 1) * QC],
                        st[:, :QC * len(g)],
                        Exp, scale=1.0 / D,
                    )
                    prev = g
                    if gi == 2:
                        # First tree level over k-blocks 0..7, mid-chunk so
                        # the final-chunk tail is short.
                        nc.vector.tensor_add(
                            esum[:, :4 * QC], es[:, :4 * QC],
                            es[:, 4 * QC:8 * QC])
                for kb in prev:
                    nc.tensor.matmul(
                        yT,
                        lhsT=vs[:, kb * 128:(kb + 1) * 128],
                        rhs=es[:, kb * QC:(kb + 1) * QC],
                        start=(kb == 0), stop=(kb == NKB - 1),
                    )
                # y^T PSUM -> SBUF (cast to bf16 for fast PE transposes).
                ytsb = ytsp.tile([128, QC], bf16, tag="ytsb", name=f"ytsb_{j}_{qc}")
                nc.vector.tensor_copy(ytsb, yT)
                # Remaining tree levels (second half + halvings down to QC).
                # The 128-partition remainder is reduced in fp32 on the PE,
                # which averages out the bf16 rounding.
                nc.vector.tensor_add(es[:, 8 * QC:12 * QC],
                                     es[:, 8 * QC:12 * QC],
                                     es[:, 12 * QC:16 * QC])
                nc.vector.tensor_add(esum[:, :4 * QC], esum[:, :4 * QC],
                                     es[:, 8 * QC:12 * QC])
                nc.vector.tensor_add(esum[:, :2 * QC], esum[:, :2 * QC],
                                     esum[:, 2 * QC:4 * QC])
                nc.vector.tensor_add(esum[:, :QC], esum[:, :QC],
                                     esum[:, QC:2 * QC])
                return {"esum": esum, "ytsb": ytsb, "j": j, "qc": qc}

            def emit_B(state):
                """Denominator + reciprocal + transpose + scale + store."""
                j, qc = state["j"], state["qc"]
                esum, ytsb = state["esum"], state["ytsb"]
                nqb = QC // 128
                dcol = auxp.tile([128, nqb], f32, tag="aux",
                                 name=f"dcol_{j}_{qc}")
                for qb in range(nqb):
                    nc.tensor.matmul(
                        dcol[:, qb:qb + 1],
                        lhsT=esum[:, qb * 128:(qb + 1) * 128],
                        rhs=ones,
                        start=True, stop=True,
                    )
                rall = rallp.tile([128, nqb], f32, tag="rall", name=f"rall_{j}_{qc}")
                nc.vector.reciprocal(rall, dcol)
                yt_t = auxp.tile([128, QC], bf16, tag="aux",
                                 name=f"ytt_{j}_{qc}")
                for qb in range(nqb):
                    nc.tensor.transpose(
                        yt_t[:, qb * 128:(qb + 1) * 128],
                        ytsb[:, qb * 128:(qb + 1) * 128],
                        ident,
                    )
                ynt = ynp.tile([128, QC], f32, tag="yn", name=f"yn_{j}_{qc}")
                for qb in range(nqb):
                    nc.vector.tensor_scalar_mul(
                        ynt[:, qb * 128:(qb + 1) * 128],
                        yt_t[:, qb * 128:(qb + 1) * 128],
                        rall[:, qb:qb + 1],
                    )
                out_ap = y[j, qc * QC:(qc + 1) * QC, :].rearrange(
                    "(qb p) d -> p qb d", p=128)
                nc.gpsimd.dma_start(
                    out=out_ap, in_=ynt.rearrange("p (qb d) -> p qb d", qb=nqb))

            pending = []
            for j in range(n_pairs):
                kts = ktsp.tile([128, S], bf16, tag="kts", name=f"kts_{j}")
                nc.sync.dma_start(out=kts, in_=kt[j])
                # First q-chunk ahead of the rest so the PE can start early.
                qts = qtsp.tile([128, S], bf16, tag="qts", name=f"qts_{j}")
                nc.sync.dma_start(out=qts[:, :QC], in_=qt[j][:, :QC])
                vs = vsp.tile([128, NKB * 128], bf16, tag="vs", name=f"vs_{j}")
                nc.sync.dma_start(
                    out=vs, in_=vt[j].rearrange("p t d -> p (t d)"))
                nc.sync.dma_start(out=qts[:, QC:], in_=qt[j][:, QC:])
                tiles = {"qkv": (qts, kts, vs)}
                for qc in range(nqc):
                    a_state = emit_A(j, qc, tiles)
                    pending.append(a_state)
                    # Defer tail work one chunk so its PE/DVE instructions
                    # queue behind the next chunk's work (no engine stalls
                    # waiting on the DVE tree-sum).
                    if len(pending) > 1:
                        emit_B(pending.pop(0))
            for state in pending:
                emit_B(state)

    nc.compile()
    return nc


def _get_nc(n_pairs=PAIRS, nqc=S // QC):
    key = (n_pairs, nqc)
    if key not in _cache:
        _cache[key] = _build(n_pairs, nqc)
    return _cache[key]


def _shard_inputs(q, k, v):
    """Build per-core input maps. Core c handles b = c // 4 and heads
    [(c % 4) * 4, (c % 4) * 4 + 4)."""
    bf16 = ml_dtypes.bfloat16
    q = np.asarray(q, dtype=np.float32)
    k = np.asarray(k, dtype=np.float32)
    v = np.asarray(v, dtype=np.float32)
    in_maps = []
    for c in range(N_CORES):
        b = c // (N_CORES // B)
        h0 = (c % (N_CORES // B)) * PAIRS
        qs = q[b, :, h0:h0 + PAIRS, :]  # [S, PAIRS, D]
        ks = k[b, :, h0:h0 + PAIRS, :]
        vs = v[b, :, h0:h0 + PAIRS, :]
        qt = np.ascontiguousarray(qs.transpose(1, 2, 0)).astype(bf16)  # [P, D, S]
        kt = np.ascontiguousarray(ks.transpose(1, 2, 0)).astype(bf16)
        # [P, kpos_local, kb, d]: per-partition lines contiguous in DRAM.
        vt = np.ascontiguousarray(
            vs.transpose(1, 0, 2).reshape(PAIRS, NKB, 128, 128)
            .transpose(0, 2, 1, 3)).astype(bf16)
        in_maps.append({"qt": qt, "kt": kt, "vt": vt})
    return in_maps


def _assemble(results):
    y_full = np.empty((B, S, H, D), dtype=np.float32)
    for c in range(N_CORES):
        b = c // (N_CORES // B)
        h0 = (c % (N_CORES // B)) * PAIRS
        yc = results[c]["y"]  # [PAIRS, S, D]
        for j in range(PAIRS):
            y_full[b, :, h0 + j, :] = yc[j]
    return y_full.reshape(B * S, H * D)


def kernel(q, k, v):
    from concourse.bass_utils import run_bass_kernel_spmd

    nc = _get_nc()
    in_maps = _shard_inputs(q, k, v)
    res = run_bass_kernel_spmd(nc, in_maps, core_ids=list(range(N_CORES)))
    return _assemble(res.results)


# revision 6
# speedup vs baseline: 1.3863x; 1.3863x over previous
"""Trainium2 Bass kernel for multi-head attention (B=2, S=2048, H=16, D=128).

Computes y = softmax(Q @ K^T / D) @ V per (batch, head) pair, returning
[B*S, H*D] float32.

Sharding: 32 (b, h) pairs across 8 cores, 4 pairs per core (tensor parallel
over heads, data parallel over batch). Each core computes full S x S
attention for its pairs. Host pre-transposes Q/K to [d, s] layout (d-major)
and casts Q/K/V to bf16 so the device kernel needs no input transposes.

Per-core dataflow per (pair, q-chunk of 512):
  - S^T[kpos, q] = K @ Q^T via PE matmuls (lhsT=K^T block, rhs=Q^T chunk),
    accumulated in PSUM in batches of 4/2 k-blocks (ping-ponged between two
    PSUM pools sized to fill the ACT pipe with 2048/1024-elem exp ops).
  - exp(S^T / 128) on the scalar engine (scale fused into the activation),
    PSUM -> SBUF, bf16 out. No max-subtraction: |scores/128| < ~0.5 for
    randn inputs, so exp is well-conditioned.
  - y^T[d, q] += matmul (lhsT=V block [kpos, d], rhs=exp block [kpos, q])
    accumulated over the 16 k-blocks in PSUM.
  - Softmax denominator: binary-tree sum of the 16 exp blocks on DVE (bf16,
    2x mode, first level starts mid-chunk), then a PE matmul against a
    ones-vector reduces the remaining 128 partitions -> denom per q (fp32).
  - y^T copied to SBUF (cast bf16), PE-transposed per 128x128 block to
    y[q, d], scaled by 1/denom (per-partition scalar on DVE), DMA'd out.

The scalar engine (exp over S^2 elements at 1 elem/cycle/lane) is the
roofline for this kernel; the schedule keeps it saturated.
"""

import numpy as np
import ml_dtypes

B, S, H, D = 2, 2048, 16, 128
N_CORES = 8
PAIRS = (B * H) // N_CORES  # 4 pairs per core
QC = 512                    # q-chunk size
NKB = S // 128              # 16 k-blocks per sequence
# k-block batches per q-chunk: the score pool is [128, 3*QC] x 2 slots
# (6 PSUM banks); slot-reuse distance 2 keeps the scalar engine fed across
# group and chunk boundaries while yT (1 bank) + aux (1 bank) fill PSUM.
GROUPS = [[0, 1, 2], [3, 4, 5], [6, 7, 8], [9, 10, 11], [12, 13], [14, 15]]

_cache = {}


def _build(n_pairs, nqc):
    import concourse.bacc as bacc
    import concourse.tile as tile
    import concourse.mybir as mybir
    from concourse.masks import make_identity

    bf16 = mybir.dt.bfloat16
    f32 = mybir.dt.float32
    Exp = mybir.ActivationFunctionType.Exp

    nc = bacc.Bacc(None, target_bir_lowering=False, debug=False)
    qt = nc.dram_tensor("qt", [n_pairs, 128, S], bf16, kind="ExternalInput")
    kt = nc.dram_tensor("kt", [n_pairs, 128, S], bf16, kind="ExternalInput")
    vt = nc.dram_tensor("vt", [n_pairs, 128, NKB, 128], bf16, kind="ExternalInput")
    y = nc.dram_tensor("y", [n_pairs, S, 128], f32, kind="ExternalOutput")

    with tile.TileContext(nc) as tc:
        with (
            tc.tile_pool(name="const", bufs=1) as constp,
            tc.tile_pool(name="qts", bufs=2) as qtsp,
            tc.tile_pool(name="kts", bufs=2) as ktsp,
            tc.tile_pool(name="vs", bufs=2) as vsp,
            tc.tile_pool(name="es", bufs=2) as esp,
            tc.tile_pool(name="esum", bufs=2) as esump,
            tc.tile_pool(name="yts", bufs=2) as ytsp,
            tc.tile_pool(name="rall", bufs=2) as rallp,
            tc.tile_pool(name="yn", bufs=3) as ynp,
            tc.tile_pool(name="st", bufs=2, space="PSUM") as stp,
            tc.tile_pool(name="yT", bufs=1, space="PSUM") as yTp,
            tc.tile_pool(name="aux", bufs=1, space="PSUM") as auxp,
        ):
            ones = constp.tile([128, 1], bf16)
            nc.vector.memset(ones, 1.0)
            ident = constp.tile([128, 128], bf16)
            make_identity(nc, ident)

            def emit_A(j, qc, tiles):
                """Score matmuls + exp + y^T accumulation + yT copy + tree-sum."""
                qts, kts, vs = tiles["qkv"]
                es = esp.tile([128, NKB * QC], bf16, tag="es", name=f"es_{j}_{qc}")
                esum = esump.tile([128, NKB * QC // 4], bf16,
                                  tag="esum", name=f"esum_{j}_{qc}")
                yT = yTp.tile([128, QC], f32, tag="yT", name=f"yT_{j}_{qc}")
                q_sl = qts[:, qc * QC:(qc + 1) * QC]
                prev = None
                for gi, g in enumerate(GROUPS):
                    st = stp.tile([128, QC * len(g)], f32, tag="st",
                                  name=f"st_{j}_{qc}_{g[0]}")
                    for i, kb in enumerate(g):
                        nc.tensor.matmul(
                            st[:, i * QC:(i + 1) * QC],
                            lhsT=kts[:, kb * 128:(kb + 1) * 128],
                            rhs=q_sl,
                            start=True, stop=True,
                        )
                    # y-matmuls of the previous group keep PE busy while the
                    # scalar engine runs exp on this group.
                    if prev is not None:
                        for kb in prev:
                            nc.tensor.matmul(
                                yT,
                                lhsT=vs[:, kb * 128:(kb + 1) * 128],
                                rhs=es[:, kb * QC:(kb + 1) * QC],
                                start=(kb == 0), stop=(kb == NKB - 1),
                            )
                    nc.scalar.activation(
                        es[:, g[0] * QC:(g[-1] + 1) * QC],
                        st[:, :QC * len(g)],
                        Exp, scale=1.0 / D,
                    )
                    prev = g
                    if gi == 2:
                        # First tree level over k-blocks 0..7, mid-chunk so
                        # the final-chunk tail is short.
                        nc.vector.tensor_add(
                            esum[:, :4 * QC], es[:, :4 * QC],
                            es[:, 4 * QC:8 * QC])
                for kb in prev:
                    nc.tensor.matmul(
                        yT,
                        lhsT=vs[:, kb * 128:(kb + 1) * 128],
                        rhs=es[:, kb * QC:(kb + 1) * QC],
                        start=(kb == 0), stop=(kb == NKB - 1),
                    )
                # y^T PSUM -> SBUF (cast to bf16 for fast PE transposes).
                ytsb = ytsp.tile([128, QC], bf16, tag="ytsb", name=f"ytsb_{j}_{qc}")
                nc.vector.tensor_copy(ytsb, yT)
                # Remaining tree levels (second half + halvings down to QC).
                # The 128-partition remainder is reduced in fp32 on the PE,
                # which averages out the bf16 rounding.
                nc.vector.tensor_add(es[:, 8 * QC:12 * QC],
                                     es[:, 8 * QC:12 * QC],
                                     es[:, 12 * QC:16 * QC])
                nc.vector.tensor_add(esum[:, :4 * QC], esum[:, :4 * QC],
                                     es[:, 8 * QC:12 * QC])
                nc.vector.tensor_add(esum[:, :2 * QC], esum[:, :2 * QC],
                                     esum[:, 2 * QC:4 * QC])
                nc.vector.tensor_add(esum[:, :QC], esum[:, :QC],
                                     esum[:, QC:2 * QC])
                return {"esum": esum, "ytsb": ytsb, "j": j, "qc": qc}

            def emit_B(state):
                """Denominator + reciprocal + transpose + scale + store."""
                j, qc = state["j"], state["qc"]
                esum, ytsb = state["esum"], state["ytsb"]
                nqb = QC // 128
                dcol = auxp.tile([128, nqb], f32, tag="aux",
                                 name=f"dcol_{j}_{qc}")
                for qb in range(nqb):
                    nc.tensor.matmul(
                        dcol[:, qb:qb + 1],
                        lhsT=esum[:, qb * 128:(qb + 1) * 128],
                        rhs=ones,
                        start=True, stop=True,
                    )
                rall = rallp.tile([128, nqb], f32, tag="rall", name=f"rall_{j}_{qc}")
                nc.vector.reciprocal(rall, dcol)
                yt_t = auxp.tile([128, QC], bf16, tag="aux",
                                 name=f"ytt_{j}_{qc}")
                for qb in range(nqb):
                    nc.tensor.transpose(
                        yt_t[:, qb * 128:(qb + 1) * 128],
                        ytsb[:, qb * 128:(qb + 1) * 128],
                        ident,
                    )
                ynt = ynp.tile([128, QC], f32, tag="yn", name=f"yn_{j}_{qc}")
                for qb in range(nqb):
                    nc.vector.tensor_scalar_mul(
                        ynt[:, qb * 128:(qb + 1) * 128],
                        yt_t[:, qb * 128:(qb + 1) * 128],
                        rall[:, qb:qb + 1],
                    )
                out_ap = y[j, qc * QC:(qc + 1) * QC, :].rearrange(
                    "(qb p) d -> p qb d", p=128)
                nc.gpsimd.dma_start(
                    out=out_ap, in_=ynt.rearrange("p (qb d) -> p qb d", qb=nqb))

            pending = []
            for j in range(n_pairs):
                kts = ktsp.tile([128, S], bf16, tag="kts", name=f"kts_{j}")
                nc.sync.dma_start(out=kts, in_=kt[j])
                # First q-chunk ahead of the rest so the PE can start early.
                qts = qtsp.tile([128, S], bf16, tag="qts", name=f"qts_{j}")
                nc.sync.dma_start(out=qts[:, :QC], in_=qt[j][:, :QC])
                vs = vsp.tile([128, NKB * 128], bf16, tag="vs", name=f"vs_{j}")
                nc.sync.dma_start(
                    out=vs, in_=vt[j].rearrange("p t d -> p (t d)"))
                nc.sync.dma_start(out=qts[:, QC:], in_=qt[j][:, QC:])
                tiles = {"qkv": (qts, kts, vs)}
                for qc in range(nqc):
                    a_state = emit_A(j, qc, tiles)
                    pending.append(a_state)
                    # Defer tail work one chunk so its PE/DVE instructions
                    # queue behind the next chunk's work (no engine stalls
                    # waiting on the DVE tree-sum).
                    if len(pending) > 1:
                        emit_B(pending.pop(0))
            for state in pending:
                emit_B(state)

    nc.compile()
    return nc


def _get_nc(n_pairs=PAIRS, nqc=S // QC):
    key = (n_pairs, nqc)
    if key not in _cache:
        _cache[key] = _build(n_pairs, nqc)
    return _cache[key]


def _shard_inputs(q, k, v):
    """Build per-core input maps. Core c handles b = c // 4 and heads
    [(c % 4) * 4, (c % 4) * 4 + 4)."""
    bf16 = ml_dtypes.bfloat16
    q = np.asarray(q, dtype=np.float32)
    k = np.asarray(k, dtype=np.float32)
    v = np.asarray(v, dtype=np.float32)
    in_maps = []
    for c in range(N_CORES):
        b = c // (N_CORES // B)
        h0 = (c % (N_CORES // B)) * PAIRS
        qs = q[b, :, h0:h0 + PAIRS, :]  # [S, PAIRS, D]
        ks = k[b, :, h0:h0 + PAIRS, :]
        vs = v[b, :, h0:h0 + PAIRS, :]
        qt = np.ascontiguousarray(qs.transpose(1, 2, 0)).astype(bf16)  # [P, D, S]
        kt = np.ascontiguousarray(ks.transpose(1, 2, 0)).astype(bf16)
        # [P, kpos_local, kb, d]: per-partition lines contiguous in DRAM.
        vt = np.ascontiguousarray(
            vs.transpose(1, 0, 2).reshape(PAIRS, NKB, 128, 128)
            .transpose(0, 2, 1, 3)).astype(bf16)
        in_maps.append({"qt": qt, "kt": kt, "vt": vt})
    return in_maps


def _assemble(results):
    y_full = np.empty((B, S, H, D), dtype=np.float32)
    for c in range(N_CORES):
        b = c // (N_CORES // B)
        h0 = (c % (N_CORES // B)) * PAIRS
        yc = results[c]["y"]  # [PAIRS, S, D]
        for j in range(PAIRS):
            y_full[b, :, h0 + j, :] = yc[j]
    return y_full.reshape(B * S, H * D)


def kernel(q, k, v):
    from concourse.bass_utils import run_bass_kernel_spmd

    nc = _get_nc()
    in_maps = _shard_inputs(q, k, v)
    res = run_bass_kernel_spmd(nc, in_maps, core_ids=list(range(N_CORES)))
    return _assemble(res.results)
